# revision 10
# baseline (speedup 1.0000x reference)
"""KoLeo-loss kernel for 8 Trainium2 NeuronCores.

Reference computation (for x of shape [B=16384, D=256] f32):
    xn   = x / ||x||_row                       (L2 row normalize)
    gram = xn @ xn.T
    min_dist_i = min_{j != i} sqrt(clip(2 - 2*gram_ij, 0))
    loss = -mean(log(min_dist + 1e-8))

Device strategy (one identical SPMD program on 8 cores):
  - Core c receives xr = roll(x, -c*2048, axis=0): its 2048 query rows are
    local rows 0..2047, and the self-match (diagonal) of local query m sits
    at local column m.  Row-max is permutation invariant, so rolling is free.
  - Phase A: load 128-row chunks, row-normalize in f32 (ACT square+accum,
    ACT sqrt, DVE reciprocal, DVE scale+cast to fp16), PE-transpose into a
    feature-major fp16 tile xT [128p(feature), 2(k), n_rows].
  - Phase B: for each 128-query chunk (stationary = slice of xT), stream all
    database columns through the PE in 512-col PSUM banks (K=256 as two
    accumulated passes).  Drain: ACT copies half the banks PSUM->SBUF f32;
    DVE tensor_tensor_reduce(max, max) consumes (psum bank, sbuf copy) pairs
    and maintains the running row max in a [128,1] accumulator.  Self-match
    is killed by adding -4 to the one 512-col bank holding the diagonal.
  - Output per core: gmax [128, 16] f32 (row-max of gram per query).
Host finishes: min_dist = sqrt(2-2*gmax), loss = -mean(log(min_dist+1e-8)).
"""

import sys

if "/opt/trn_rl_repo" not in sys.path:
    sys.path.insert(0, "/opt/trn_rl_repo")

import numpy as np

D = 256
P = 128
BANK = 512  # psum bank width in f32 elements
SPAN = 8  # psum banks in flight per span
B_FULL = 16384
N_CORES = 8
QPC = B_FULL // N_CORES  # queries per core


def make_dmask() -> np.ndarray:
    """dmask[p, t, j] = -4 where j == t*128+p else 0.

    Query chunk mc (local rows mc*128+p) has its self-match in bank mc//4
    at in-bank column (mc%4)*128 + p; tile t = mc%4 kills it.
    """
    dm = np.zeros((P, 4, BANK), dtype=np.float32)
    for t in range(4):
        for p in range(P):
            dm[p, t, t * P + p] = -4.0
    return dm


def build_nc(n_rows: int, n_q: int):
    import concourse.mybir as mybir
    import concourse.tile as tile
    from concourse import bacc
    from concourse.masks import make_identity

    dt = mybir.dt
    AF = mybir.ActivationFunctionType
    OP = mybir.AluOpType

    assert n_rows % (BANK * SPAN) == 0
    assert n_q % P == 0
    n_mc = n_q // P
    n_chunks = n_rows // P
    n_groups = n_chunks // 4
    n_banks = n_rows // BANK
    n_spans = n_banks // SPAN
    assert n_mc <= 4 * SPAN, "diag bank must land in span 0"

    nc = bacc.Bacc(None)
    x_in = nc.declare_dram_parameter("x", [n_rows, D], dt.float32, isOutput=False)
    dm_in = nc.declare_dram_parameter("dmask", [P, 4, BANK], dt.float32, isOutput=False)
    out_d = nc.declare_dram_parameter("gmax", [P, n_mc], dt.float32, isOutput=True)

    with tile.TileContext(nc) as tc:
        with (
            tc.tile_pool(name="persist", bufs=1) as persist,
            tc.tile_pool(name="ld", bufs=3) as ldp,
            tc.tile_pool(name="norm", bufs=4) as normp,
            tc.tile_pool(name="cp", bufs=10) as cpp,
            tc.tile_pool(name="mxp", bufs=2) as mxp,
            tc.tile_pool(name="ps", bufs=8, space="PSUM") as psp,
        ):
            xT = persist.tile([P, 2, n_rows], dt.float16)
            ident = persist.tile([P, P], dt.float16)
            make_identity(nc, ident)
            dmask = persist.tile([P, 4, BANK], dt.float32)
            nc.gpsimd.dma_start(out=dmask, in_=dm_in[:, :, :])
            gmax = persist.tile([P, n_mc], dt.float32)

            # ---------------- Phase A: normalize + transpose ----------------
            xv = x_in[:, :].rearrange("(g c p) d -> g p c d", c=4, p=P)
            for g in range(n_groups):
                xa = ldp.tile([P, 4, D], dt.float32, tag="xa")
                nc.gpsimd.dma_start(out=xa, in_=xv[g])
                n2 = normp.tile([P, 4], dt.float32, tag="n2")
                sq = normp.tile([P, D], dt.float16, tag="sq")
                for c in range(4):
                    nc.scalar.activation(
                        out=sq,
                        in_=xa[:, c, :],
                        func=AF.Square,
                        accum_out=n2[:, c : c + 1],
                    )
                nrm = normp.tile([P, 4], dt.float32, tag="nrm")
                nc.scalar.sqrt(nrm, n2)
                rn = normp.tile([P, 4], dt.float32, tag="rn")
                nc.vector.reciprocal(rn, nrm)
                xn = normp.tile([P, 4, D], dt.float16, tag="xn")
                for c in range(4):
                    nc.vector.tensor_tensor(
                        xn[:, c, :],
                        xa[:, c, :],
                        rn[:, c : c + 1].to_broadcast([P, D]),
                        OP.mult,
                    )
                for c in range(4):
                    pst = psp.tile([P, 2, P], dt.float16, tag="ps")
                    for k in range(2):
                        nc.tensor.transpose(
                            pst[:, k, :], xn[:, c, k * P : (k + 1) * P], ident
                        )
                    s = g * 4 + c
                    dst = xT[:, :, s * P : (s + 1) * P]
                    if c % 2 == 0:
                        nc.vector.tensor_copy(dst, pst)
                    else:
                        nc.scalar.copy(dst, pst)

            # ---------------- Phase B: gram row-max ----------------
            # Per span: ACT copies banks 0..6 PSUM->SBUF fp16 (2x mode); DVE
            # folds them + bank 7 (psum operand) with a fp16 TT-max tree into
            # the per-mc running max macc [128, 512]; one tensor_reduce per mc.
            def ttmax(name, a, b, sp_tag, out_ap=None):
                if out_ap is None:
                    out_ap = cpp.tile(
                        [P, BANK], dt.float16, tag=sp_tag, bufs=3, name=name
                    )
                nc.vector.tensor_tensor(out_ap, a, b, OP.max)
                return out_ap

            for mc in range(n_mc):
                diag_bank = mc // 4
                mask_t = mc % 4
                macc = mxp.tile([P, BANK], dt.float16, tag="macc")
                for sp in range(n_spans):
                    pst = [
                        psp.tile([P, BANK], dt.float32, tag="ps", name=f"psb{j}")
                        for j in range(SPAN)
                    ]
                    for k in range(2):
                        lhs = xT[:, k, mc * P : (mc + 1) * P]
                        for j in range(SPAN):
                            nb = sp * SPAN + j
                            nc.tensor.matmul(
                                pst[j],
                                lhs,
                                xT[:, k, nb * BANK : (nb + 1) * BANK],
                                start=(k == 0),
                                stop=(k == 1),
                            )
                    cps = []
                    for j in range(SPAN - 1):
                        cpt = cpp.tile(
                            [P, BANK], dt.float16, tag="cp", bufs=16, name=f"cp{j}"
                        )
                        nc.scalar.copy(cpt, pst[j])
                        cps.append(cpt)
                    if sp == 0:
                        nc.vector.tensor_tensor(
                            cps[diag_bank],
                            cps[diag_bank],
                            dmask[:, mask_t, :],
                            OP.add,
                        )
                    t0 = ttmax("t0", cps[0], cps[1], "t0")
                    t1 = ttmax("t1", cps[2], cps[3], "t1")
                    t2 = ttmax("t2", cps[4], cps[5], "t2")
                    u = ttmax("u", t0, t1, "u")
                    v = ttmax("v", pst[SPAN - 1], t2, "v")
                    uv = ttmax("uv", u, v, "uv", out_ap=(macc if sp == 0 else None))
                    if sp > 0:
                        w2 = ttmax("w2", cps[6], uv, "w2")
                        nc.vector.tensor_tensor(macc, w2, macc, OP.max)
                    else:
                        nc.vector.tensor_tensor(macc, cps[6], macc, OP.max)
                nc.vector.tensor_reduce(
                    gmax[:, mc : mc + 1], macc, axis=mybir.AxisListType.X, op=OP.max
                )

            nc.sync.dma_start(out=out_d[:, :], in_=gmax)

    nc.compile()
    return nc


_NC_CACHE = {}


def _get_nc(n_rows, n_q):
    key = (n_rows, n_q)
    if key not in _NC_CACHE:
        _NC_CACHE[key] = build_nc(n_rows, n_q)
    return _NC_CACHE[key]


LAST_RESULT = None  # BassKernelResults of the most recent run (for profiling)


def kernel(student_output: np.ndarray) -> np.ndarray:
    import os

    from concourse.bass_utils import run_bass_kernel_spmd

    global LAST_RESULT
    x = np.ascontiguousarray(student_output, dtype=np.float32)
    assert x.shape == (B_FULL, D)

    nc = _get_nc(B_FULL, QPC)
    dm = make_dmask()
    in_maps = [
        {"x": np.roll(x, -c * QPC, axis=0), "dmask": dm} for c in range(N_CORES)
    ]
    trace = bool(int(os.environ.get("KOLEO_TRACE", "0")))
    res = run_bass_kernel_spmd(
        nc, in_maps, core_ids=list(range(N_CORES)), trace=trace
    )
    LAST_RESULT = res

    gmax = np.empty(B_FULL, dtype=np.float32)
    for c in range(N_CORES):
        gm = res.results[c]["gmax"]  # [128, n_mc]
        gmax[c * QPC : (c + 1) * QPC] = gm.T.ravel()

    min_dist = np.sqrt(np.clip(2.0 - 2.0 * gmax.astype(np.float64), 0.0, None))
    loss = -np.mean(np.log(min_dist + 1e-8))
    return np.float32(loss)


if __name__ == "__main__":
    rng = np.random.default_rng(0)
    x = rng.standard_normal((B_FULL, D), dtype=np.float32)
    out = kernel(x)
    print("loss:", out)


# revision 13
# speedup vs baseline: 1.1096x; 1.1096x over previous
"""KoLeo-loss kernel for 8 Trainium2 NeuronCores.

Reference computation (for x of shape [B=16384, D=256] f32):
    xn   = x / ||x||_row                       (L2 row normalize)
    gram = xn @ xn.T
    min_dist_i = min_{j != i} sqrt(clip(2 - 2*gram_ij, 0))
    loss = -mean(log(min_dist + 1e-8))

Device strategy (one identical SPMD program on 8 cores):
  - Core c receives xr = roll(x, -c*2048, axis=0): its 2048 query rows are
    local rows 0..2047, and the self-match (diagonal) of local query m sits
    at local column m.  Row-max is permutation invariant, so rolling is free.
  - Phase A: load 128-row chunks, row-normalize in f32 (ACT square+accum,
    ACT sqrt, DVE reciprocal, DVE scale+cast to fp16), PE-transpose into a
    feature-major fp16 tile xT [128p(feature), 2(k), n_rows].
  - Phase B: for each 128-query chunk (stationary = slice of xT), stream all
    database columns through the PE in 512-col PSUM banks (K=256 as two
    accumulated passes).  Drain: ACT copies half the banks PSUM->SBUF f32;
    DVE tensor_tensor_reduce(max, max) consumes (psum bank, sbuf copy) pairs
    and maintains the running row max in a [128,1] accumulator.  Self-match
    is killed by adding -4 to the one 512-col bank holding the diagonal.
  - Output per core: gmax [128, 16] f32 (row-max of gram per query).
Host finishes: min_dist = sqrt(2-2*gmax), loss = -mean(log(min_dist+1e-8)).
"""

import sys

if "/opt/trn_rl_repo" not in sys.path:
    sys.path.insert(0, "/opt/trn_rl_repo")

import numpy as np

D = 256
P = 128
BANK = 512  # psum bank width in f32 elements
SPAN = 8  # psum banks in flight per span
B_FULL = 16384
N_CORES = 8
QPC = B_FULL // N_CORES  # queries per core


def make_dmask() -> np.ndarray:
    """dmask[p, t, j] = -4 where j == t*128+p else 0.

    Query chunk mc (local rows mc*128+p) has its self-match in bank mc//4
    at in-bank column (mc%4)*128 + p; tile t = mc%4 kills it.
    """
    dm = np.zeros((P, 4, BANK), dtype=np.float32)
    for t in range(4):
        for p in range(P):
            dm[p, t, t * P + p] = -4.0
    return dm


def build_nc(n_rows: int, n_q: int):
    import concourse.mybir as mybir
    import concourse.tile as tile
    from concourse import bacc
    from concourse.masks import make_identity

    dt = mybir.dt
    AF = mybir.ActivationFunctionType
    OP = mybir.AluOpType

    assert n_rows % (BANK * SPAN) == 0
    assert n_q % P == 0
    n_mc = n_q // P
    n_chunks = n_rows // P
    n_groups = n_chunks // 4
    n_banks = n_rows // BANK
    n_spans = n_banks // SPAN
    assert n_mc <= 4 * SPAN, "diag bank must land in span 0"

    nc = bacc.Bacc(None)
    x_in = nc.declare_dram_parameter("x", [n_rows, D], dt.float32, isOutput=False)
    dm_in = nc.declare_dram_parameter("dmask", [P, 4, BANK], dt.float32, isOutput=False)
    out_d = nc.declare_dram_parameter("gmax", [P, n_mc], dt.float32, isOutput=True)

    with tile.TileContext(nc) as tc:
        with (
            tc.tile_pool(name="persist", bufs=1) as persist,
            tc.tile_pool(name="ld", bufs=3) as ldp,
            tc.tile_pool(name="norm", bufs=4) as normp,
            tc.tile_pool(name="cp", bufs=10) as cpp,
            tc.tile_pool(name="mxp", bufs=2) as mxp,
            tc.tile_pool(name="ps", bufs=8, space="PSUM") as psp,
        ):
            xT = persist.tile([P, 2, n_rows], dt.float16)
            ident = persist.tile([P, P], dt.float16)
            make_identity(nc, ident)
            dmask = persist.tile([P, 4, BANK], dt.float32)
            nc.gpsimd.dma_start(out=dmask, in_=dm_in[:, :, :])
            gmax = persist.tile([P, n_mc], dt.float32)

            # ---------------- Phase A: normalize + transpose ----------------
            xv = x_in[:, :].rearrange("(g c p) d -> g p c d", c=4, p=P)
            for g in range(n_groups):
                xa = ldp.tile([P, 4, D], dt.float32, tag="xa")
                nc.gpsimd.dma_start(out=xa, in_=xv[g])
                n2 = normp.tile([P, 4], dt.float32, tag="n2")
                sq = normp.tile([P, D], dt.float16, tag="sq")
                for c in range(4):
                    nc.scalar.activation(
                        out=sq,
                        in_=xa[:, c, :],
                        func=AF.Square,
                        accum_out=n2[:, c : c + 1],
                    )
                nrm = normp.tile([P, 4], dt.float32, tag="nrm")
                nc.scalar.sqrt(nrm, n2)
                rn = normp.tile([P, 4], dt.float32, tag="rn")
                nc.vector.reciprocal(rn, nrm)
                xn = normp.tile([P, 4, D], dt.float16, tag="xn")
                for c in range(4):
                    nc.vector.tensor_tensor(
                        xn[:, c, :],
                        xa[:, c, :],
                        rn[:, c : c + 1].to_broadcast([P, D]),
                        OP.mult,
                    )
                # Transpose via NORMAL matmul (out = xn_half.T @ I): faster
                # than transpose-mode and keeps the HAM activity monitor warm.
                # Two chunks share one psum tile so the drain copy runs FD=512.
                for cc in range(2):
                    pst = psp.tile([P, 2, 2 * P], dt.float32, tag="ps")
                    for ci in range(2):
                        c = 2 * cc + ci
                        for k in range(2):
                            nc.tensor.matmul(
                                pst[:, k, ci * P : (ci + 1) * P],
                                xn[:, c, k * P : (k + 1) * P],
                                ident,
                                start=True,
                                stop=True,
                            )
                    s = g * 4 + 2 * cc
                    dst = xT[:, :, s * P : (s + 2) * P]
                    nc.vector.tensor_copy(dst, pst)

            # ---------------- Phase B: gram row-max ----------------
            # Per span: ACT copies banks 0..6 PSUM->SBUF fp16 (2x mode); DVE
            # folds them + bank 7 (psum operand) with a fp16 TT-max tree into
            # the per-mc running max macc [128, 512]; one tensor_reduce per mc.
            def ttmax(name, a, b, sp_tag, out_ap=None):
                if out_ap is None:
                    out_ap = cpp.tile(
                        [P, BANK], dt.float16, tag=sp_tag, bufs=3, name=name
                    )
                nc.vector.tensor_tensor(out_ap, a, b, OP.max)
                return out_ap

            for mc in range(n_mc):
                diag_bank = mc // 4
                mask_t = mc % 4
                macc = mxp.tile([P, BANK], dt.float16, tag="macc")
                for sp in range(n_spans):
                    pst = [
                        psp.tile([P, BANK], dt.float32, tag="ps", name=f"psb{j}")
                        for j in range(SPAN)
                    ]
                    for k in range(2):
                        lhs = xT[:, k, mc * P : (mc + 1) * P]
                        for j in range(SPAN):
                            nb = sp * SPAN + j
                            nc.tensor.matmul(
                                pst[j],
                                lhs,
                                xT[:, k, nb * BANK : (nb + 1) * BANK],
                                start=(k == 0),
                                stop=(k == 1),
                            )
                    cps = []
                    for j in range(SPAN - 2):
                        cpt = cpp.tile(
                            [P, BANK], dt.float16, tag="cp", bufs=16, name=f"cp{j}"
                        )
                        nc.scalar.copy(cpt, pst[j])
                        cps.append(cpt)
                    if sp == 0:
                        nc.vector.tensor_tensor(
                            cps[diag_bank],
                            cps[diag_bank],
                            dmask[:, mask_t, :],
                            OP.add,
                        )
                    a = ttmax("a", pst[SPAN - 2], cps[0], "a")
                    b = ttmax("b", pst[SPAN - 1], cps[1], "b")
                    u = ttmax("u", cps[2], cps[3], "u")
                    v = ttmax("v", cps[4], cps[5], "v")
                    s_ = ttmax("s_", a, b, "s_")
                    t_ = ttmax("t_", u, v, "t_")
                    if sp == 0:
                        ttmax("uv", s_, t_, "uv", out_ap=macc)
                    else:
                        w2 = ttmax("w2", s_, t_, "w2")
                        nc.vector.tensor_tensor(macc, w2, macc, OP.max)
                nc.vector.tensor_reduce(
                    gmax[:, mc : mc + 1], macc, axis=mybir.AxisListType.X, op=OP.max
                )

            nc.sync.dma_start(out=out_d[:, :], in_=gmax)

    nc.compile()
    return nc


_NC_CACHE = {}


def _get_nc(n_rows, n_q):
    key = (n_rows, n_q)
    if key not in _NC_CACHE:
        _NC_CACHE[key] = build_nc(n_rows, n_q)
    return _NC_CACHE[key]


LAST_RESULT = None  # BassKernelResults of the most recent run (for profiling)


def kernel(student_output: np.ndarray) -> np.ndarray:
    import os

    from concourse.bass_utils import run_bass_kernel_spmd

    global LAST_RESULT
    x = np.ascontiguousarray(student_output, dtype=np.float32)
    assert x.shape == (B_FULL, D)

    nc = _get_nc(B_FULL, QPC)
    dm = make_dmask()
    in_maps = [
        {"x": np.roll(x, -c * QPC, axis=0), "dmask": dm} for c in range(N_CORES)
    ]
    trace = bool(int(os.environ.get("KOLEO_TRACE", "0")))
    res = run_bass_kernel_spmd(
        nc, in_maps, core_ids=list(range(N_CORES)), trace=trace
    )
    LAST_RESULT = res

    gmax = np.empty(B_FULL, dtype=np.float32)
    for c in range(N_CORES):
        gm = res.results[c]["gmax"]  # [128, n_mc]
        gmax[c * QPC : (c + 1) * QPC] = gm.T.ravel()

    min_dist = np.sqrt(np.clip(2.0 - 2.0 * gmax.astype(np.float64), 0.0, None))
    loss = -np.mean(np.log(min_dist + 1e-8))
    return np.float32(loss)


if __name__ == "__main__":
    rng = np.random.default_rng(0)
    x = rng.standard_normal((B_FULL, D), dtype=np.float32)
    out = kernel(x)
    print("loss:", out)


# revision 14
# speedup vs baseline: 1.1514x; 1.0376x over previous
"""KoLeo-loss kernel for 8 Trainium2 NeuronCores.

Reference computation (for x of shape [B=16384, D=256] f32):
    xn   = x / ||x||_row                       (L2 row normalize)
    gram = xn @ xn.T
    min_dist_i = min_{j != i} sqrt(clip(2 - 2*gram_ij, 0))
    loss = -mean(log(min_dist + 1e-8))

Device strategy (one identical SPMD program on 8 cores):
  - Core c receives xr = roll(x, -c*2048, axis=0): its 2048 query rows are
    local rows 0..2047, and the self-match (diagonal) of local query m sits
    at local column m.  Row-max is permutation invariant, so rolling is free.
  - Phase A: load 128-row chunks, row-normalize in f32 (ACT square+accum,
    ACT sqrt, DVE reciprocal, DVE scale+cast to fp16), PE-transpose into a
    feature-major fp16 tile xT [128p(feature), 2(k), n_rows].
  - Phase B: for each 128-query chunk (stationary = slice of xT), stream all
    database columns through the PE in 512-col PSUM banks (K=256 as two
    accumulated passes).  Drain: ACT copies half the banks PSUM->SBUF f32;
    DVE tensor_tensor_reduce(max, max) consumes (psum bank, sbuf copy) pairs
    and maintains the running row max in a [128,1] accumulator.  Self-match
    is killed by adding -4 to the one 512-col bank holding the diagonal.
  - Output per core: gmax [128, 16] f32 (row-max of gram per query).
Host finishes: min_dist = sqrt(2-2*gmax), loss = -mean(log(min_dist+1e-8)).
"""

import sys

if "/opt/trn_rl_repo" not in sys.path:
    sys.path.insert(0, "/opt/trn_rl_repo")

import numpy as np

D = 256
P = 128
BANK = 512  # psum bank width in f32 elements
SPAN = 8  # psum banks in flight per span
B_FULL = 16384
N_CORES = 8
QPC = B_FULL // N_CORES  # queries per core


def make_dmask() -> np.ndarray:
    """dmask[p, t, j] = -4 where j == t*128+p else 0.

    Query chunk mc (local rows mc*128+p) has its self-match in bank mc//4
    at in-bank column (mc%4)*128 + p; tile t = mc%4 kills it.
    """
    dm = np.zeros((P, 4, BANK), dtype=np.float32)
    for t in range(4):
        for p in range(P):
            dm[p, t, t * P + p] = -4.0
    return dm


def build_nc(n_rows: int, n_q: int):
    import concourse.mybir as mybir
    import concourse.tile as tile
    from concourse import bacc
    from concourse.masks import make_identity

    dt = mybir.dt
    AF = mybir.ActivationFunctionType
    OP = mybir.AluOpType

    assert n_rows % (BANK * SPAN) == 0
    assert n_q % P == 0
    n_mc = n_q // P
    n_chunks = n_rows // P
    n_groups = n_chunks // 4
    n_banks = n_rows // BANK
    n_spans = n_banks // SPAN
    assert n_mc <= 4 * SPAN, "diag bank must land in span 0"

    nc = bacc.Bacc(None)
    x_in = nc.declare_dram_parameter("x", [n_rows, D], dt.float32, isOutput=False)
    dm_in = nc.declare_dram_parameter("dmask", [P, 4, BANK], dt.float32, isOutput=False)
    out_d = nc.declare_dram_parameter("gmax", [P, n_mc], dt.float32, isOutput=True)

    PAIR = 2 * BANK  # two psum banks per tile: fewer, bigger drain ops

    with tile.TileContext(nc) as tc:
        with (
            tc.tile_pool(name="persist", bufs=1) as persist,
            tc.tile_pool(name="ld", bufs=4) as ldp,
            tc.tile_pool(name="norm", bufs=6) as normp,
            tc.tile_pool(name="cp", bufs=8) as cpp,
            tc.tile_pool(name="mxp", bufs=2) as mxp,
            tc.tile_pool(name="ps", bufs=4, space="PSUM") as psp,
        ):
            xT = persist.tile([P, 2, n_rows], dt.float16)
            ident = persist.tile([P, P], dt.float16)
            make_identity(nc, ident)
            dmask = persist.tile([P, 4, BANK], dt.float32)
            nc.gpsimd.dma_start(out=dmask, in_=dm_in[:, :, :])
            gmax = persist.tile([P, n_mc], dt.float32)

            def ttmax(name, a, b, out_ap=None):
                if out_ap is None:
                    out_ap = cpp.tile(
                        [P, PAIR], dt.float16, tag=name, bufs=3, name=name
                    )
                nc.vector.tensor_tensor(out_ap, a, b, OP.max)
                return out_ap

            # One span: 8 banks as 4 psum pair-tiles. ACT copies 3 pairs to
            # fp16; DVE consumes the 4th pair as a psum TT operand and folds
            # everything into the per-mc running max macc [128, 1024] fp16.
            def emit_span(mc, sp, macc):
                pt = [
                    psp.tile([P, PAIR], dt.float32, tag="ps", name=f"pp{j}")
                    for j in range(4)
                ]
                for k in range(2):
                    lhs = xT[:, k, mc * P : (mc + 1) * P]
                    for j in range(4):
                        for h in range(2):
                            nb = sp * SPAN + 2 * j + h
                            nc.tensor.matmul(
                                pt[j][:, h * BANK : (h + 1) * BANK],
                                lhs,
                                xT[:, k, nb * BANK : (nb + 1) * BANK],
                                start=(k == 0),
                                stop=(k == 1),
                            )
                cps = []
                for j in range(3):
                    cpt = cpp.tile(
                        [P, PAIR], dt.float16, tag="cp", bufs=8, name=f"cp{j}"
                    )
                    nc.scalar.copy(cpt, pt[j])
                    cps.append(cpt)
                if sp == 0:
                    db = mc // 4  # diagonal bank: tile db//2, half db%2
                    tdx, h = db // 2, db % 2
                    seg = cps[tdx][:, h * BANK : (h + 1) * BANK]
                    nc.vector.tensor_tensor(seg, seg, dmask[:, mc % 4, :], OP.add)
                a = ttmax("a", pt[3], cps[0])
                u = ttmax("u", cps[1], cps[2])
                if sp == 0:
                    ttmax("uv", a, u, out_ap=macc)
                else:
                    w = ttmax("w", a, u)
                    nc.vector.tensor_tensor(macc, w, macc, OP.max)

            def finish_mc(mc, macc):
                nc.vector.tensor_reduce(
                    gmax[:, mc : mc + 1], macc, axis=mybir.AxisListType.X, op=OP.max
                )

            # ---------------- PE warmup burst (HAM un-throttle) -------------
            wps = psp.tile([P, P], dt.float32, tag="ps", name="warm")
            for _ in range(24):
                nc.tensor.matmul(wps, ident, ident, start=True, stop=True)

            # ---------------- Phase A: normalize + transpose ----------------
            # mc=0's spans are interleaved: span sp only needs banks
            # 8sp..8sp+7 = groups 8sp..8sp+7, so it runs as soon as they land.
            macc0 = mxp.tile([P, PAIR], dt.float16, tag="macc", name="macc0")
            xv = x_in[:, :].rearrange("(g c p) d -> g p c d", c=4, p=P)
            for g in range(n_groups):
                xa = ldp.tile([P, 4, D], dt.float32, tag="xa")
                nc.gpsimd.dma_start(out=xa, in_=xv[g])
                n2 = normp.tile([P, 4], dt.float32, tag="n2")
                sq = normp.tile([P, D], dt.float16, tag="sq")
                for c in range(4):
                    nc.scalar.activation(
                        out=sq,
                        in_=xa[:, c, :],
                        func=AF.Square,
                        accum_out=n2[:, c : c + 1],
                    )
                nrm = normp.tile([P, 4], dt.float32, tag="nrm")
                nc.scalar.sqrt(nrm, n2)
                rn = normp.tile([P, 4], dt.float32, tag="rn")
                nc.vector.reciprocal(rn, nrm)
                xn = normp.tile([P, 4, D], dt.float16, tag="xn")
                for c in range(4):
                    nc.vector.tensor_tensor(
                        xn[:, c, :],
                        xa[:, c, :],
                        rn[:, c : c + 1].to_broadcast([P, D]),
                        OP.mult,
                    )
                # Transpose via NORMAL matmul (out = xn_half.T @ I): faster
                # than transpose-mode and counts as PE activity for HAM.
                # Two chunks share one psum tile so the drain copy runs FD=512.
                for cc in range(2):
                    pst = psp.tile([P, 2, 2 * P], dt.float32, tag="ps")
                    for ci in range(2):
                        c = 2 * cc + ci
                        for k in range(2):
                            nc.tensor.matmul(
                                pst[:, k, ci * P : (ci + 1) * P],
                                xn[:, c, k * P : (k + 1) * P],
                                ident,
                                start=True,
                                stop=True,
                            )
                    s = g * 4 + 2 * cc
                    dst = xT[:, :, s * P : (s + 2) * P]
                    nc.vector.tensor_copy(dst, pst)
                if g % 8 == 7 and (g // 8) < n_spans:
                    emit_span(0, g // 8, macc0)
            finish_mc(0, macc0)

            # ---------------- Phase B: remaining query chunks ---------------
            for mc in range(1, n_mc):
                macc = mxp.tile([P, PAIR], dt.float16, tag="macc")
                for sp in range(n_spans):
                    emit_span(mc, sp, macc)
                finish_mc(mc, macc)

            nc.sync.dma_start(out=out_d[:, :], in_=gmax)

    nc.compile()
    return nc


_NC_CACHE = {}


def _get_nc(n_rows, n_q):
    key = (n_rows, n_q)
    if key not in _NC_CACHE:
        _NC_CACHE[key] = build_nc(n_rows, n_q)
    return _NC_CACHE[key]


LAST_RESULT = None  # BassKernelResults of the most recent run (for profiling)


def kernel(student_output: np.ndarray) -> np.ndarray:
    import os

    from concourse.bass_utils import run_bass_kernel_spmd

    global LAST_RESULT
    x = np.ascontiguousarray(student_output, dtype=np.float32)
    assert x.shape == (B_FULL, D)

    nc = _get_nc(B_FULL, QPC)
    dm = make_dmask()
    in_maps = [
        {"x": np.roll(x, -c * QPC, axis=0), "dmask": dm} for c in range(N_CORES)
    ]
    trace = bool(int(os.environ.get("KOLEO_TRACE", "0")))
    res = run_bass_kernel_spmd(
        nc, in_maps, core_ids=list(range(N_CORES)), trace=trace
    )
    LAST_RESULT = res

    gmax = np.empty(B_FULL, dtype=np.float32)
    for c in range(N_CORES):
        gm = res.results[c]["gmax"]  # [128, n_mc]
        gmax[c * QPC : (c + 1) * QPC] = gm.T.ravel()

    min_dist = np.sqrt(np.clip(2.0 - 2.0 * gmax.astype(np.float64), 0.0, None))
    loss = -np.mean(np.log(min_dist + 1e-8))
    return np.float32(loss)


if __name__ == "__main__":
    rng = np.random.default_rng(0)
    x = rng.standard_normal((B_FULL, D), dtype=np.float32)
    out = kernel(x)
    print("loss:", out)
